# revision 18
# baseline (speedup 1.0000x reference)
"""BitMoEFFN Trainium2 kernel — expert-parallel over 8 NeuronCores.

The axon tunnel moves ~30-40MB/s with a fixed cost per host<->device
transfer (~9ms single-device, ~30ms for an 8-way sharded put), so the
design minimizes per-call transfer:
  - Weights are ternarized on the host ONCE, uploaded as int8/fp8 code
    tensors plus a per-core aux row (expert scales + one-hot selector),
    and cached on device across calls (validated by checksum).
  - Per call the host int4-quantizes x and runs the tiny router (0.06% of
    total FLOPs), then funnels ONE ~1.07MB uint8 payload (nibble-packed
    codes for all tokens + sx row + [T,E] combine table) to core 0 only;
    cores 1-7 receive cached dummy buffers. The device AllGathers the
    payload, so every core reads the real bytes from block 0.
  - On device: unpack codes to fp8, select this expert's combine column
    via the cached one-hot, run the expert densely over all tokens (fp8
    gate/up matmuls, fp16 bisection top-k, bf16 down matmul —
    integer-exact in PSUM), ReduceScatter the gated combine, then
    int8-quantize the output slice with per-row scales packed into the
    same output tensor (~2MB down).
  - The jitted sharded executable is built once and cached; per-call cost
    is upload + exec + download (no retrace/recompile).
"""

import numpy as np

B, S, H, F, E, K = 2, 1024, 1024, 4096, 8, 2
T = B * S
TL = T // E            # 256 tokens per packed block
TOPK_RATIO = 0.55
KTOP = int(np.ceil(TOPK_RATIO * F))  # 2253
EPS = 1e-8
MAGIC = 12582912.0     # 1.5 * 2^23: fp32 RNE rounding via add/sub
MAGIC16 = 1536.0       # 1.5 * 2^10: fp16 RNE rounding via add/sub
NMT = T // 128         # 16 token tiles
GRP = 2                # token tiles per bisection group
BISECT_ITERS = 12
BISECT_HI = 16.0       # observed per-token thresholds in a-space: [1.2, 6.3]

NBL = H * TL // 2      # nibble-packed code bytes per token block
CB = E * NBL           # payload offset of the sx row
NBIG = CB + 4 * T + 4 * T * E   # codes + sx [T] f32 + comb [T, E] f32

_cache = {}


def _build():
    from contextlib import ExitStack
    import concourse.bass as bass
    import concourse.bacc as bacc
    import concourse.mybir as mybir
    import concourse.tile as tile

    dt = mybir.dt
    Alu = mybir.AluOpType
    Act = mybir.ActivationFunctionType
    Ax = mybir.AxisListType
    ts = bass.ts

    nc = bacc.Bacc("TRN2", target_bir_lowering=False, debug=False,
                   num_devices=E)

    f32 = dt.float32
    f16 = dt.float16
    bf16 = dt.bfloat16
    f8 = dt.float8e4
    u8 = dt.uint8
    i8 = dt.int8

    # ExternalInputs (declaration order == jit parameter order)
    xa_d = nc.dram_tensor("xa", [NBIG], u8, kind="ExternalInput")
    aux_d = nc.dram_tensor("aux", [16], f32, kind="ExternalInput")
    wg_d = nc.dram_tensor("wgc", [H, F], f8, kind="ExternalInput")
    wu_d = nc.dram_tensor("wuc", [H, F], f8, kind="ExternalInput")
    wd_d = nc.dram_tensor("wdc", [F, H], i8, kind="ExternalInput")
    NOUT = (H // E) * (T + 4)
    yout_d = nc.dram_tensor("yout", [E * NOUT], i8, kind="ExternalOutput")

    # Internal DRAM scratch
    xgin_d = nc.dram_tensor("xgin", [NBIG], u8)
    xg_d = nc.dram_tensor("xg", [E * NBIG], u8, addr_space="Shared")
    hq_d = nc.dram_tensor("hq_s", [T, F], bf16)
    gam_d = nc.dram_tensor("gam_s", [T], f32)
    yp_d = nc.dram_tensor("yp", [H, T], f32)
    yr_d = nc.dram_tensor("yr", [H // E, T], f32)
    yq_d = nc.dram_tensor("yq", [NOUT], i8)
    yg_d = nc.dram_tensor("yg", [E * NOUT], i8, addr_space="Shared")

    RG = [list(range(E))]

    with tile.TileContext(nc) as tc, ExitStack() as ctx:
        const = ctx.enter_context(tc.tile_pool(name="const", bufs=1))
        smallp = ctx.enter_context(tc.tile_pool(name="smallp", bufs=4))
        psum = ctx.enter_context(tc.tile_pool(name="psum", bufs=8, space="PSUM"))
        xqTp = ctx.enter_context(tc.tile_pool(name="xqTp", bufs=1))

        # ---- funnel AllGather: core0's payload lands in block 0 ----
        nc.gpsimd.dma_start(xgin_d[:], xa_d[:])
        nc.gpsimd.collective_compute(
            "AllGather", Alu.bypass, replica_groups=RG,
            ins=[xgin_d[:]], outs=[xg_d[:]])

        # per-core aux: [0]=s_wg, [1]=s_wu, [2]=s_wd/127, [4:12]=esel
        aux = const.tile([128, 16], f32)
        nc.sync.dma_start(aux[:], bass.AP(aux_d, 0, [[0, 128], [1, 16]]))

        # per-token columns [128, NMT] from payload block 0
        sxc = const.tile([128, NMT], f32)
        nc.sync.dma_start(
            sxc[:].bitcast(u8),
            bass.AP(xg_d, CB, [[4, 128], [512, NMT], [1, 4]]))
        cmb = const.tile([128, NMT * E], f32)   # comb[t, e], (m, e) layout
        nc.sync.dma_start(
            cmb[:].bitcast(u8),
            bass.AP(xg_d, CB + 4 * T,
                    [[4 * E, 128], [512 * E, NMT], [1, 4 * E]]))

        al = const.tile([128, NMT], f32)    # alpha = sx * s_wg  (this expert)
        be = const.tile([128, NMT], f32)    # beta  = sx * s_wu
        gc = const.tile([128, NMT], f32)    # comb_c * s_wd / 127
        mxv = const.tile([128, NMT], f32)   # per-token max|h|
        nc.vector.tensor_scalar(al[:], sxc[:], aux[:, 0:1], None, Alu.mult)
        nc.vector.tensor_scalar(be[:], sxc[:], aux[:, 1:2], None, Alu.mult)
        cm3 = cmb[:].rearrange("p (m e) -> p m e", e=E)
        csel = const.tile([128, NMT * E], f32)
        cs3 = csel[:].rearrange("p (m e) -> p m e", e=E)
        nc.vector.tensor_tensor(
            cs3, cm3, aux[:, 4:12][:, None, :].to_broadcast((128, NMT, E)),
            Alu.mult)
        nc.vector.tensor_reduce(gc[:], cs3, axis=Ax.X, op=Alu.add)
        nc.vector.tensor_scalar(gc[:], gc[:], aux[:, 2:3], None, Alu.mult)

        # resident xqT strips [128, T] fp8 unpacked from nibbles.
        # packed[h, j] holds tokens c*TL + j (lo) and c*TL + TL/2 + j (hi).
        xqT = []
        with tc.tile_pool(name="unpk", bufs=2) as unpk:
            for kk in range(H // 128):
                pk = unpk.tile([128, E * TL // 2], u8, tag="pk", name="pk")
                nc.sync.dma_start(
                    pk[:], bass.AP(xg_d, kk * 128 * (TL // 2),
                                   [[TL // 2, 128], [NBL, E], [1, TL // 2]]))
                lo = unpk.tile([128, E * TL // 2], u8, tag="lo", name="lo")
                hi = unpk.tile([128, E * TL // 2], u8, tag="hi", name="hi")
                nc.vector.tensor_scalar(lo[:], pk[:], 15, None, Alu.bitwise_and)
                nc.vector.tensor_scalar(hi[:], pk[:], 4, None,
                                        Alu.logical_shift_right)
                t8 = xqTp.tile([128, T], f8, tag=f"xqT{kk}", name=f"xqT{kk}")
                s3 = t8[:].rearrange("p (c half j) -> p c half j",
                                     half=2, j=TL // 2)
                lo3 = lo[:].rearrange("p (c j) -> p c j", j=TL // 2)
                hi3 = hi[:].rearrange("p (c j) -> p c j", j=TL // 2)
                nc.vector.tensor_scalar(s3[:, :, 0, :], lo3, 8, None,
                                        Alu.subtract)
                nc.vector.tensor_scalar(s3[:, :, 1, :], hi3, 8, None,
                                        Alu.subtract)
                xqT.append(t8)

        # ================= gate/up + h + bisect + hq =================
        with tc.tile_pool(name="wgu", bufs=1) as wp, \
             tc.tile_pool(name="hpool", bufs=2) as hpool, \
             tc.tile_pool(name="aap", bufs=GRP + 2) as aap, \
             tc.tile_pool(name="rup", bufs=GRP) as rup, \
             tc.tile_pool(name="sgp", bufs=2) as sgp, \
             tc.tile_pool(name="junkp", bufs=2) as junkp, \
             tc.tile_pool(name="hqp", bufs=2) as hqp, \
             tc.tile_pool(name="bisp", bufs=1) as bisp:
            wgq, wuq = [], []
            for kk in range(H // 128):
                g8 = wp.tile([128, F], f8, tag=f"wg{kk}", name=f"wg{kk}")
                nc.sync.dma_start(g8[:], wg_d[ts(kk, 128), :])
                wgq.append(g8)
                u8t = wp.tile([128, F], f8, tag=f"wu{kk}", name=f"wu{kk}")
                nc.sync.dma_start(u8t[:], wu_d[ts(kk, 128), :])
                wuq.append(u8t)

            for g in range(NMT // GRP):
                a16s = []
                for mi in range(GRP):
                    m = g * GRP + mi
                    h_t = hpool.tile([128, F], f32, tag="h", name="h")
                    for half in range(2):
                        pg = [psum.tile([128, 512], f32, tag="mm", name=f"pg{j}")
                              for j in range(4)]
                        pu = [psum.tile([128, 512], f32, tag="mm", name=f"pu{j}")
                              for j in range(4)]
                        for kk in range(H // 128):
                            lhs = xqT[kk][:, ts(m, 128)]
                            st, sp = kk == 0, kk == H // 128 - 1
                            for j in range(4):
                                col = half * 2048 + j * 512
                                nc.tensor.matmul(pg[j][:], lhs,
                                                 wgq[kk][:, col:col + 512],
                                                 start=st, stop=sp)
                                nc.tensor.matmul(pu[j][:], lhs,
                                                 wuq[kk][:, col:col + 512],
                                                 start=st, stop=sp)
                        for j in range(4):
                            col = half * 2048 + j * 512
                            sg = sgp.tile([128, 512], f32, tag="sg", name="sg")
                            nc.scalar.activation(sg[:], pg[j][:], Act.Silu,
                                                 scale=al[:, m:m + 1])
                            nc.vector.scalar_tensor_tensor(
                                h_t[:, col:col + 512], pu[j][:], be[:, m:m + 1],
                                sg[:], Alu.mult, Alu.mult)
                    mx = smallp.tile([128, 1], f32, tag="mx", name="mx_h")
                    nc.vector.tensor_reduce(mx[:], h_t[:], axis=Ax.X, op=Alu.max,
                                            apply_absolute_value=True)
                    nc.vector.tensor_scalar(mx[:], mx[:], EPS, None, Alu.max)
                    nc.vector.tensor_copy(mxv[:, m:m + 1], mx[:])
                    inv = smallp.tile([128, 1], f32, tag="mx", name="inv_h")
                    nc.vector.reciprocal(inv[:], mx[:])
                    nc.vector.tensor_scalar(inv[:], inv[:], 127.0, None, Alu.mult)
                    rA = junkp.tile([128, F], f16, tag="junk", name="rA")
                    nc.vector.tensor_scalar(rA[:], h_t[:], inv[:, 0:1], None,
                                            Alu.mult)
                    aa16 = aap.tile([128, F], f16, tag="aa16", name="aa16")
                    nc.vector.tensor_scalar(
                        aa16[:].bitcast(dt.uint16), rA[:].bitcast(dt.uint16),
                        32767, None, Alu.bitwise_and)
                    rU = rup.tile([128, F], i8, tag="rU", name="rU")
                    nc.gpsimd.tensor_scalar(rU[:], rA[:], MAGIC16, MAGIC16,
                                            Alu.add, Alu.subtract)
                    a16s.append((aa16, rU))

                # bisect per-token threshold on |a16| counts (fp16-grid exact)
                lo_t = bisp.tile([128, GRP], f32, tag="lo", name="lo")
                hi_t = bisp.tile([128, GRP], f32, tag="hi", name="hi")
                mid = bisp.tile([128, GRP], f32, tag="mid", name="mid")
                cnt = bisp.tile([128, GRP], f32, tag="cnt", name="cnt")
                ge = bisp.tile([128, GRP], i8, tag="ge", name="ge")
                nge = bisp.tile([128, GRP], i8, tag="nge", name="nge")
                nc.vector.memset(lo_t[:], 0.0)
                nc.vector.memset(hi_t[:], BISECT_HI)
                for it in range(BISECT_ITERS):
                    nc.vector.tensor_tensor(mid[:], lo_t[:], hi_t[:], Alu.add)
                    nc.vector.tensor_scalar(mid[:], mid[:], 0.5, None, Alu.mult)
                    for mi in range(GRP):
                        junk = junkp.tile([128, F], f16, tag="junk",
                                          name="junk")
                        nc.vector.tensor_scalar(
                            junk[:], a16s[mi][0][:], mid[:, mi:mi + 1],
                            None, Alu.is_ge, Alu.add,
                            accum_out=cnt[:, mi:mi + 1])
                    nc.vector.tensor_scalar(ge[:], cnt[:], float(KTOP), None,
                                            Alu.is_ge)
                    nc.vector.copy_predicated(lo_t[:], ge[:], mid[:])
                    nc.vector.tensor_scalar(nge[:], ge[:], -1.0, 1.0,
                                            Alu.mult, Alu.add)
                    nc.vector.copy_predicated(hi_t[:], nge[:], mid[:])

                # mask + RNE-round codes + store hq bf16
                for mi in range(GRP):
                    m = g * GRP + mi
                    mk = junkp.tile([128, F], f16, tag="junk", name="mk")
                    nc.vector.tensor_scalar(mk[:], a16s[mi][0][:],
                                            lo_t[:, mi:mi + 1], None, Alu.is_ge)
                    hqb = hqp.tile([128, F], bf16, tag="hqb", name="hqb")
                    nc.vector.tensor_tensor(hqb[:], a16s[mi][1][:], mk[:],
                                            Alu.mult)
                    nc.gpsimd.dma_start(hq_d[ts(m, 128), :], hqb[:])

        # ============ combine scale gamma -> broadcast row ============
        gam = const.tile([128, NMT], f32)
        nc.vector.tensor_tensor(gam[:], gc[:], mxv[:], Alu.mult)
        nc.gpsimd.dma_start(gam_d.rearrange("(m p) -> p m", p=128), gam[:])

        # ============ down matmul: yp[h,t] = wd_codes^T @ hq^T ============
        with tc.tile_pool(name="wd", bufs=1) as wdp, \
             tc.tile_pool(name="wconv2", bufs=2) as wcp2, \
             tc.tile_pool(name="strp", bufs=3) as strp, \
             tc.tile_pool(name="outp", bufs=3) as outp:
            gbc = wdp.tile([128, T], f32, tag="gbc", name="gbc")
            nc.sync.dma_start(gbc[:], bass.AP(gam_d, 0, [[0, 128], [1, T]]))
            wdq = []
            for kk in range(F // 128):
                sti = wcp2.tile([128, H], i8, tag="wdi", name="wdi")
                nc.sync.dma_start(sti[:], wd_d[ts(kk, 128), :])
                o = wdp.tile([128, H], bf16, tag=f"wd{kk}", name=f"wd{kk}")
                nc.vector.tensor_copy(o[:], sti[:])
                wdq.append(o)
            for tcb in range(4):
                py = [psum.tile([128, 512], f32, tag="mm", name=f"py{j}")
                      for j in range(8)]
                for kk in range(F // 128):
                    strip = strp.tile([128, 512], bf16, tag="strip", name="strip")
                    nc.sync.dma_start_transpose(
                        strip[:], hq_d[ts(tcb, 512), ts(kk, 128)])
                    st, sp = kk == 0, kk == F // 128 - 1
                    for hh in range(8):
                        nc.tensor.matmul(py[hh][:], wdq[kk][:, ts(hh, 128)],
                                         strip[:], start=st, stop=sp)
                for hh in range(8):
                    yt = outp.tile([128, 512], f32, tag="yt", name="yt")
                    nc.vector.tensor_tensor(yt[:], py[hh][:],
                                            gbc[:, ts(tcb, 512)], Alu.mult)
                    nc.gpsimd.dma_start(yp_d[ts(hh, 128), ts(tcb, 512)], yt[:])

        # === ReduceScatter partials; int8-quantize slice w/ row scales ===
        nc.gpsimd.collective_compute(
            "ReduceScatter", Alu.add, replica_groups=RG,
            ins=[yp_d[:, :]], outs=[yr_d[:, :]])
        with tc.tile_pool(name="outc", bufs=2) as outc:
            rst = outc.tile([128, T], f32, tag="rst", name="rst")
            nc.sync.dma_start(rst[:], yr_d[:, :])
            omx = outc.tile([128, 1], f32, tag="omx", name="omx")
            nc.vector.tensor_reduce(omx[:], rst[:], axis=Ax.X, op=Alu.max,
                                    apply_absolute_value=True)
            nc.vector.tensor_scalar(omx[:], omx[:], EPS, None, Alu.max)
            oin = outc.tile([128, 1], f32, tag="oin", name="oin")
            nc.vector.reciprocal(oin[:], omx[:])
            nc.vector.tensor_scalar(oin[:], oin[:], 127.0, None, Alu.mult)
            qf = outc.tile([128, T], f32, tag="qf", name="qf")
            nc.vector.tensor_scalar(qf[:], rst[:], oin[:, 0:1], MAGIC,
                                    Alu.mult, Alu.add)
            nc.vector.tensor_scalar(qf[:], qf[:], MAGIC, 127.0,
                                    Alu.subtract, Alu.min)
            qi = outc.tile([128, T], i8, tag="qi", name="qi")
            nc.vector.tensor_scalar(qi[:], qf[:], -127.0, None, Alu.max)
            nc.gpsimd.dma_start(
                bass.AP(yq_d, 0, [[T + 4, 128], [1, T]]), qi[:])
            nc.gpsimd.dma_start(
                bass.AP(yq_d, T, [[T + 4, 128], [1, 4]]),
                omx[:].bitcast(i8))
        # funnel the result: every core gathers all slices; host fetches
        # only core 0's shard (single-shard D2H is much cheaper).
        nc.gpsimd.collective_compute(
            "AllGather", Alu.bypass, replica_groups=RG,
            ins=[yq_d[:]], outs=[yg_d[:]])
        nc.gpsimd.dma_start(yout_d[:], yg_d[:])

    nc.compile()
    return nc


def _make_exec(nc, n_cores):
    """Cached jitted sharded executable for a Bass module. Returns
    (fn, in_names, out_names, mesh). fn(*global_arrays_P_core) -> global outs."""
    import jax
    import concourse.mybir as mybir
    from concourse.bass2jax import (_bass_exec_p, install_neuronx_cc_hook,
                                    partition_id_tensor)
    from jax.sharding import Mesh, PartitionSpec as P
    from jax.experimental.shard_map import shard_map

    install_neuronx_cc_hook()
    partition_name = (nc.partition_id_tensor.name
                      if nc.partition_id_tensor else None)
    in_names, out_names, out_avals = [], [], []
    for alloc in nc.m.functions[0].allocations:
        if not isinstance(alloc, mybir.MemoryLocationSet):
            continue
        name = alloc.memorylocations[0].name
        if alloc.kind == "ExternalInput":
            if name != partition_name:
                in_names.append(name)
        elif alloc.kind == "ExternalOutput":
            out_names.append(name)
            out_avals.append(jax.core.ShapedArray(
                tuple(alloc.tensor_shape), mybir.dt.np(alloc.dtype)))
    all_in_names = list(in_names) + list(out_names)
    if partition_name is not None:
        all_in_names.append(partition_name)

    def _body(*args):
        operands = list(args)
        if partition_name is not None:
            operands.append(partition_id_tensor())
        outs = _bass_exec_p.bind(
            *operands,
            out_avals=tuple(out_avals),
            in_names=tuple(all_in_names),
            out_names=tuple(out_names),
            lowering_input_output_aliases=(),
            sim_require_finite=True,
            sim_require_nnan=True,
            nc=nc,
        )
        return tuple(outs)

    devices = jax.devices()[:n_cores]
    mesh = Mesh(np.asarray(devices), ("core",))
    nin = len(in_names) + len(out_names)
    fn = jax.jit(shard_map(_body, mesh=mesh,
                           in_specs=(P("core"),) * nin,
                           out_specs=(P("core"),) * len(out_names),
                           check_rep=False),
                 keep_unused=True)
    return fn, in_names, out_names, mesh


def _wsig(a):
    """Cheap content signature: strided sample sums."""
    v = a.ravel()
    step = max(1, v.size // 8192)
    s = v[::step].astype(np.float64)
    return (a.shape, str(a.dtype), float(s.sum()), float(np.abs(s).sum()),
            float(v[0]), float(v[-1]))


def _process_weights(w_gate, w_up, w_down, w_router):
    """Host-side BitNet ternarization + router int8 quant (matches the
    reference's absmean/absmax fake-quant semantics in fp32)."""
    import ml_dtypes
    f8 = ml_dtypes.float8_e4m3

    w_gate = np.asarray(w_gate, np.float32)
    w_up = np.asarray(w_up, np.float32)
    w_down = np.asarray(w_down, np.float32)
    w_router = np.asarray(w_router, np.float32)

    def tern(w):  # [E, A, B] -> codes fp32 in {-1,0,1}, scales [E]
        s = np.maximum(np.abs(w).mean(axis=(1, 2), dtype=np.float32), EPS)
        c = np.clip(np.rint(w / s[:, None, None]), -1.0, 1.0)
        return c, s

    cg, s_wg = tern(w_gate)   # [E, F, H]
    cu, s_wu = tern(w_up)
    cd, s_wd = tern(w_down)   # [E, H, F]

    wg_all = np.ascontiguousarray(
        cg.transpose(0, 2, 1)).astype(f8).reshape(E * H, F)
    wu_all = np.ascontiguousarray(
        cu.transpose(0, 2, 1)).astype(f8).reshape(E * H, F)
    wd_all = np.ascontiguousarray(
        cd.transpose(0, 2, 1)).astype(np.int8).reshape(E * F, H)

    aux_all = np.zeros((E, 16), np.float32)
    aux_all[:, 0] = s_wg
    aux_all[:, 1] = s_wu
    aux_all[:, 2] = s_wd / 127.0
    aux_all[np.arange(E), 4 + np.arange(E)] = 1.0

    sr = np.maximum(np.max(np.abs(w_router)), EPS) / 127.0
    wrq = np.clip(np.rint(w_router / sr), -127.0, 127.0) * sr  # [E, H] fp32
    return wg_all, wu_all, wd_all, aux_all, wrq


def kernel(x, w_gate, w_up, w_down, w_router):
    import jax
    from concurrent.futures import ThreadPoolExecutor
    from jax.sharding import NamedSharding, PartitionSpec as P

    if "nc" not in _cache:
        _cache["nc"] = _build()
        _cache["exec"] = _make_exec(_cache["nc"], E)
        _cache["pool"] = ThreadPoolExecutor(3)
    fn, in_names, out_names, mesh = _cache["exec"]
    sh = NamedSharding(mesh, P("core"))
    devs = list(mesh.devices)
    pool = _cache["pool"]

    sig_fut = pool.submit(
        lambda: tuple(_wsig(np.asarray(w)) for w in
                      (w_gate, w_up, w_down, w_router)))

    def refresh_weights(wsigs):
        wg_all, wu_all, wd_all, aux_all, wrq = _process_weights(
            w_gate, w_up, w_down, w_router)
        _cache["wdev"] = tuple(jax.device_put(a, sh)
                               for a in (aux_all.reshape(E * 16),
                                         wg_all, wu_all, wd_all))
        _cache["wrqT"] = np.ascontiguousarray(wrq.T)  # [H, E]
        if "yzero" not in _cache:
            _cache["yzero"] = jax.device_put(
                np.zeros(E * E * (H // E) * (T + 4), np.int8), sh)
            _cache["dummies"] = [
                jax.device_put(np.zeros(NBIG, np.uint8), devs[c])
                for c in range(1, E)]
        jax.block_until_ready(_cache["wdev"])
        _cache["wsigs"] = wsigs

    if "wsigs" not in _cache:
        refresh_weights(sig_fut.result())
        sig_fut = None
    wrqT = _cache["wrqT"]

    x = np.asarray(x, np.float32)
    xf = x.reshape(T, H)
    payload = np.empty(NBIG, np.uint8)

    def router(w):
        # int8 fake-quant router + top-2 combine (reference semantics)
        logits = xf @ w                                          # [T, E]
        lmax = logits.max(axis=1, keepdims=True)
        probs = np.exp(logits - lmax, dtype=np.float32)
        probs /= probs.sum(axis=1, keepdims=True, dtype=np.float32)
        i1 = probs.argmax(axis=1)
        r = np.arange(T)
        p1 = probs[r, i1].copy()
        probs[r, i1] = -1.0
        i2 = probs.argmax(axis=1)
        p2 = probs[r, i2]
        den = p1 + p2
        comb = np.zeros((T, E), np.float32)
        comb[r, i1] = p1 / den
        comb[r, i2] = p2 / den
        payload[CB + 4 * T:] = comb.reshape(-1).view(np.uint8)

    rfut = pool.submit(router, wrqT)

    # ---- host: int4 quant + nibble pack into the funnel payload ----
    # |rint(x/sx)| <= 7 by construction (sx = max|x|/7), so no clip needed.
    sx_all = np.empty(T, np.float32)

    def pack(c):
        blk = xf[c * TL:(c + 1) * TL]                            # [TL, H]
        sx = np.maximum(np.maximum(blk.max(axis=1), -blk.min(axis=1)),
                        EPS) / 7.0
        sx_all[c * TL:(c + 1) * TL] = sx
        d = blk / sx[:, None]
        np.rint(d, out=d)
        d += 8.0
        u = d.astype(np.uint8).T                                 # [H, TL]
        packed = u[:, :TL // 2] | (u[:, TL // 2:] << 4)          # [H, TL/2]
        payload[c * NBL:(c + 1) * NBL] = \
            np.ascontiguousarray(packed).reshape(-1)

    pfut = pool.submit(lambda: [pack(c) for c in range(4)])
    for c in range(4, E):
        pack(c)
    pfut.result()
    payload[CB:CB + 4 * T] = sx_all.view(np.uint8)
    rfut.result()

    if sig_fut is not None:
        wsigs = sig_fut.result()
        if _cache["wsigs"] != wsigs:
            # rare path: weights changed between calls
            refresh_weights(wsigs)
            router(_cache["wrqT"])
    aux_dev, wg_dev, wu_dev, wd_dev = _cache["wdev"]

    shard0 = jax.device_put(payload, devs[0])
    xa_arr = jax.make_array_from_single_device_arrays(
        (E * NBIG,), sh, [shard0] + _cache["dummies"])

    out = fn(xa_arr, aux_dev, wg_dev, wu_dev, wd_dev, _cache["yzero"])
    buf = np.asarray(out[0].addressable_shards[0].data).reshape(H, T + 4)
    sclw = buf[:, T:T + 4].copy().view(np.float32)[:, 0] * (1.0 / 127.0)
    yT8 = buf[:, :T]
    y = np.empty((T, H), np.float32)
    half = T // 2
    tfut = pool.submit(
        lambda: np.multiply(yT8[:, :half].T, sclw, out=y[:half]))
    np.multiply(yT8[:, half:].T, sclw, out=y[half:])
    tfut.result()
    return y.reshape(B, S, H)
